# revision 54
# baseline (speedup 1.0000x reference)
"""Trainium2 Bass kernel for nn_Pooling_block (B=128, N=785, C=384, pp=2).

Pure data-parallel over batch: 16 batches per core x 8 NeuronCores.
~115 us HW exec (baseline was 230 us); rel err ~1.7e-3 (gate 2e-2).

Final design ("c-major", fp16 x / fp8 edge / fp16 out, balanced SDMA):
  - x in float16 channel-major [B, 128, 3, 784]; edge in float8-e4m3
    c-major [B, 128, 3, 788] (3 zero-pad tokens); output stored fp16 and
    upcast on host.  Per-core HBM traffic: 15.4 MB reads + 4.8 MB writes
    (vs 56 MB all-f32).  fp8 only feeds a mean -> sigmoid and is
    accumulated in f32; fp16 everywhere else keeps the error ~1.7e-3.
  - all compute is 128-partition-wide c-major (no PE transposes, no
    single-lane [1, N] ops):
      * pair sums + node sums: 3 DVE stt ops with accum_out
      * edge sums: 3 DVE stt token-half folds with accum_out
      * one sigmoid per batch covers both means ([128, 2, 3] tile; edge
        host-pre-scaled by 784/785 so a single 1/784 scale fits both)
      * group ci: 9 matmuls (W_lin.T blocks stationary)
      * scores: 6 matmuls with the ci column stride-0-BROADCAST to
        [128,128] as the stationary operand -> scores land replicated on
        all 128 partitions, keeping sigmoid/t01/pooled full-width
      * t01: ONE stt op: (sig0 + 2) + sig1 -> fp16
      * pooled: fp16 mul (t01 stride-0 broadcast across chunks) on DVE +
        add on gpsimd (DVE for tail batches) writing fp16 into acm
  - finals: 12 fp16 matmuls vs W_out_cls.T chunks; PSUM->SBUF on ACT.
  - stores: [128, 768] rows via sync HWDGE (even p//8 engine split); the
    69-row tail via gpsimd SWDGE, whose swizzle spreads <128-partition
    transfers across all 16 SDMA engines (HWDGE puts 69 rows on just 3
    engines, which was the v4 bottleneck); last batch tail via ACT HWDGE
    (DMA is idle then, latency wins).
  - failed experiments kept out: DMA-CCE accumulate for edge halves (the
    single SWDGE queue is strictly FIFO, so accum chains head-of-line
    block behind stores), >2-deep accum chains, s-add on gpsimd.
"""
import os
import sys

sys.path.insert(0, "/opt/trn_rl_repo")

import numpy as np
import ml_dtypes

import concourse.bass as bass
import concourse.tile as tile
from concourse import bacc, mybir
from concourse.bass_utils import run_bass_kernel_spmd

B, N, C = 128, 785, 384
HW = N - 1          # 784
H = 28              # grid side
HP = 14             # pooled grid side
NPATCH = HP * HP    # 196
NB = 16             # batches per core
NCORES = 8
NOUT = 1 + NPATCH   # 197
CO = 2 * C          # 768
NCH = 3             # channel chunks of 128
GRP = 4             # max batches per group
NE = 788            # padded edge tokens (2 * 394)
EQ = NE // 2        # 394: tokens per CCE-accumulated half
EH = EQ // 2        # 197: tokens per DVE fold half

F32 = mybir.dt.float32
F16 = mybir.dt.float16
F8 = mybir.dt.float8e4
ADD = mybir.AluOpType.add
MUL = mybir.AluOpType.mult
SIGMOID = mybir.ActivationFunctionType.Sigmoid
COPY = mybir.ActivationFunctionType.Copy
AXC = mybir.AxisListType.X


def build_program(w_scalars):
    """Build the per-core SPMD program. w_scalars = (w00, w01, w10, w11) when
    the per-patch weights are channel-uniform, else None (general path)."""
    nc = bacc.Bacc(None, target_bir_lowering=False, debug=False)

    x_d = nc.declare_dram_parameter("x", [NB, 128, NCH, HW], F16, isOutput=False)
    e_d = nc.declare_dram_parameter("edge", [NB, 128, NCH, NE], F8, isOutput=False)
    wlt_d = nc.declare_dram_parameter("wlt", [128, NCH, C], F16, isOutput=False)
    wct_d = nc.declare_dram_parameter("wct", [128, NCH, CO], F16, isOutput=False)
    clsc_d = nc.declare_dram_parameter("cls_cm", [128, NCH, NB], F16, isOutput=False)
    general_w = w_scalars is None
    if general_w:
        # cols 0-3: w[c, q]; cols 4-5: w[c,2j] + w[c,2j+1]
        wq_d = nc.declare_dram_parameter("wq_cm", [128, NCH, 6], F32, isOutput=False)
    # output leaves the device in fp16 (host upcasts): halves store traffic;
    # quantization (~5e-4 of abs-max) is far below the pipeline's ~1.7e-3
    out_d = nc.declare_dram_parameter("out", [NB, NOUT, CO], F16, isOutput=True)

    uniform_w = (not general_w) and len(set(w_scalars)) == 1

    with tile.TileContext(nc) as tc:
        with (
            tc.tile_pool(name="const", bufs=1) as cpool,
            tc.tile_pool(name="gx", bufs=8) as gxp,
            tc.tile_pool(name="ed", bufs=8) as edp,
            tc.tile_pool(name="apool", bufs=8) as ap,
            tc.tile_pool(name="work", bufs=4) as wk,
            tc.tile_pool(name="small", bufs=4) as sm,
            tc.tile_pool(name="sgp", bufs=2) as sgp,
            tc.tile_pool(name="acm", bufs=8) as acmp,
            tc.tile_pool(name="ost", bufs=6) as ostp,
            tc.tile_pool(name="psS", bufs=3, space="PSUM") as psS,
            tc.tile_pool(name="psC", bufs=2, space="PSUM") as psC,
            tc.tile_pool(name="psF", bufs=3, space="PSUM") as psF,
        ):
            # all constant loads queue AFTER batch 0's data (the ramp's
            # critical path); wlt still lands before the first group-ci
            # matmuls need it
            wlt_h = cpool.tile([128, NCH, C], F16)
            cls_h = cpool.tile([128, NCH, NB], F16)
            wct_h = cpool.tile([128, NCH, CO], F16)
            if general_w:
                wq_h = cpool.tile([128, NCH, 6], F32)

            group_list = [range(0, 2), range(2, 6), range(6, 10),
                          range(10, 14), range(14, 16)]
            first = True

            def pass_b(bs, acms):
                # finals + stores for one group
                for b in bs:
                    acm = acms[b]
                    for r0, rn in ((0, 128), (128, 69)):
                        stile = ostp.tile([128, CO], F16, tag="ost",
                                          name="stile")
                        for nh in range(2):
                            fo = psF.tile([128, C], F32, tag="fo", name="fo")
                            for cch in range(NCH):
                                nc.tensor.matmul(
                                    fo[0:rn, :],
                                    acm[:, cch, r0 : r0 + rn],
                                    wct_h[:, cch, C * nh : C * (nh + 1)],
                                    start=(cch == 0), stop=(cch == 2),
                                )
                            nc.scalar.copy(
                                stile[0:rn, C * nh : C * (nh + 1)], fo[0:rn, :]
                            )
                        if rn == 128:
                            # 128-row store: HWDGE splits evenly (p//8);
                            # issued from sync to keep ACT free for copies
                            nc.sync.dma_start(
                                out_d[b, r0 : r0 + rn, :], stile[0:rn, :]
                            )
                        elif b == NB - 1:
                            # tail batch: DMA system is idle by now, so the
                            # low-latency HWDGE path beats SWDGE's spread
                            nc.scalar.dma_start(
                                out_d[b, r0 : r0 + rn, :], stile[0:rn, :]
                            )
                        else:
                            # <128-row store: SWDGE swizzle spreads it across
                            # all 16 SDMA engines (HWDGE would use only 3)
                            nc.gpsimd.dma_start(
                                out_d[b, r0 : r0 + rn, :], stile[0:rn, :]
                            )

            # NOTE on pass-B ordering: issuing finals one group late (to
            # keep tail scores matmuls from queueing behind a 48-matmul
            # finals wall in the PE stream - a measured 7.2us DVE stall)
            # was tried twice.  Deferring every group measured +25us (acm
            # buffer rotation couples pass A to the previous finals), and
            # deferring only the tail groups produced a NONDETERMINISTIC
            # wrong result on hardware with no speedup.  Keep pass B
            # strictly inline per group.
            for gi, bs in enumerate(group_list):
                glen = len(bs)
                x_t, a_t = {}, {}

                # s columns for the group, c-major: [128, chunk, batch]
                sG = sgp.tile([128, NCH, GRP], F16, tag="sG")

                # -- sub-loop 1: loads + per-batch token sums -> s columns --
                for b in bs:
                    gb = b - bs[0]
                    xb = gxp.tile([128, NCH, NPATCH, 4], F16, tag="xb")
                    nc.sync.dma_start(
                        xb[:],
                        x_d[b].rearrange("p k (n q) -> p k n q", q=4),
                    )
                    x_t[b] = xb
                    eb = edp.tile([128, NCH, NE], F8, tag="eb")
                    nc.sync.dma_start(eb[:], e_d[b])

                    if first:
                        nc.sync.dma_start(wlt_h[:], wlt_d[:])
                        nc.sync.dma_start(cls_h[:], clsc_d[:])
                        nc.sync.dma_start(wct_h[:], wct_d[:])
                        if general_w:
                            nc.sync.dma_start(wq_h[:], wq_d[:])
                        first = False

                    # esns[:, 0, k] = edge sums, esns[:, 1, k] = node sums
                    esns = sm.tile([128, 2, NCH], F32, tag="esns")

                    # pair sums + node sums in one pass (3 stt ops w/ accum)
                    at = ap.tile([128, NCH, NPATCH, 2], F16, tag="at")
                    for k in range(NCH):
                        nc.vector.scalar_tensor_tensor(
                            at[:, k], xb[:, k, :, 0:2], 1.0, xb[:, k, :, 2:4],
                            MUL, ADD, accum_out=esns[:, 1, k : k + 1],
                        )
                    a_t[b] = at

                    # edge sums: fold the two token-halves, f32 accumulate.
                    # (Reducing a chunk on ACT via Copy+accum_out was tried
                    # and regressed ~20us: the load-gated op head-of-line
                    # blocks ACT's in-order stream of sigmoids and copies.)
                    ef = wk.tile([128, NCH, EQ], F16, tag="ef")
                    for k in range(NCH):
                        nc.vector.scalar_tensor_tensor(
                            ef[:, k], eb[:, k, 0:EQ], 1.0, eb[:, k, EQ:NE],
                            MUL, ADD, accum_out=esns[:, 0, k : k + 1],
                        )

                    # one sigmoid for both means (edge host-pre-scaled)
                    sg2 = sm.tile([128, 2, NCH], F32, tag="sg2")
                    nc.scalar.activation(
                        sg2[:], esns[:], SIGMOID, scale=1.0 / HW
                    )
                    nc.vector.tensor_add(
                        sG[:, :, gb : gb + 1],
                        sg2[:, 0:1, :].rearrange("p u k -> p k u"),
                        sg2[:, 1:2, :].rearrange("p u k -> p k u"),
                    )

                # -- group ci: 9 matmuls, all c-major, no transposes --
                ciG_ps = psC.tile([128, NCH, GRP], F32, tag="ciG")
                for m in range(NCH):
                    for k in range(NCH):
                        nc.tensor.matmul(
                            ciG_ps[:, m, 0:glen],
                            wlt_h[:, k, 128 * m : 128 * (m + 1)],
                            sG[:, k, 0:glen],
                            start=(k == 0), stop=(k == 2),
                        )
                ciG = sm.tile([128, NCH, GRP], F16, tag="ciG_sb")
                nc.scalar.copy(ciG[:], ciG_ps[:])

                # -- pass A: scores -> t01 -> pooled -> acm per batch --
                acms = {}
                for b in bs:
                    gb = b - bs[0]
                    xb, at = x_t[b], a_t[b]

                    # scores, replicated on all 128 partitions via a
                    # stride-0 broadcast of the ci column as lhsT
                    sigb = sm.tile([128, NPATCH, 2, 2], F16, tag="sigb")
                    for h in range(2):
                        sc_ps = psS.tile([128, 392], F32, tag="sc")
                        for k in range(NCH):
                            nc.tensor.matmul(
                                sc_ps[:],
                                ciG[:, k, gb : gb + 1].to_broadcast([128, 128]),
                                xb[:, k, 98 * h : 98 * (h + 1), :].rearrange(
                                    "p n q -> p (n q)"
                                ),
                                start=(k == 0), stop=(k == 2),
                            )
                        nc.scalar.activation(
                            sigb[:, 98 * h : 98 * (h + 1)].rearrange(
                                "p n j r -> p (n j r)"
                            ),
                            sc_ps[:], SIGMOID,
                        )

                    acm = acmp.tile([128, NCH, NOUT], F16, tag="acm")
                    nc.gpsimd.tensor_copy(acm[:, :, 0:1], cls_h[:, :, b : b + 1])
                    acms[b] = acm

                    if not general_w:
                        # t01h has a singleton chunk dim for stride-0 bcast
                        t01h = wk.tile([128, 1, NPATCH, 2], F16, tag="t01h")
                        if uniform_w and w_scalars[0] == 1.0:
                            # t01[p,j] = (sig(4p+2j) + 2) + sig(4p+2j+1)
                            nc.vector.scalar_tensor_tensor(
                                t01h[:, 0], sigb[:, :, :, 0:1], 2.0,
                                sigb[:, :, :, 1:2], ADD, ADD,
                            )
                        elif uniform_w:
                            w00 = w_scalars[0]
                            t01f = wk.tile([128, NPATCH, 2], F32, tag="t01f")
                            nc.vector.scalar_tensor_tensor(
                                t01f[:], sigb[:, :, :, 0:1], 2.0,
                                sigb[:, :, :, 1:2], ADD, ADD,
                            )
                            nc.vector.tensor_scalar(
                                t01h[:, 0], t01f[:], w00, None, MUL
                            )
                        else:
                            wv = w_scalars
                            for j in range(2):
                                vj = wk.tile([128, NPATCH], F32, tag=f"vj{j}")
                                nc.vector.tensor_scalar(
                                    vj[:], sigb[:, :, j, 1],
                                    float(wv[2 * j + 1]),
                                    float(wv[2 * j] + wv[2 * j + 1]),
                                    MUL, ADD,
                                )
                                nc.vector.scalar_tensor_tensor(
                                    t01h[:, 0, :, j], sigb[:, :, j, 0],
                                    float(wv[2 * j]), vj[:], MUL, ADD,
                                )
                        prod = wk.tile([128, NCH, NPATCH, 2], F16, tag="prod")
                        nc.vector.tensor_mul(
                            prod[:], at[:],
                            t01h[:].to_broadcast([128, NCH, NPATCH, 2]),
                        )
                        # acm add off DVE (the global bottleneck) except for
                        # the tail batches, where gpsimd's slow op + the
                        # cross-engine hop would sit on the critical path
                        acm_eng = nc.vector if b >= NB - 2 else nc.gpsimd
                        acm_eng.tensor_add(
                            acm[:, :, 1:NOUT].rearrange(
                                "p k (n u) -> p k n u", u=1
                            ),
                            prod[:, :, :, 0:1], prod[:, :, :, 1:2],
                        )
                    else:
                        # fully general per-channel weights: m_j[c, p] =
                        # w[c,2j]*sig_j0 + w[c,2j+1]*sig_j1 + wsum_j[c]
                        for k in range(NCH):
                            mj = []
                            for j in range(2):
                                u = wk.tile([128, NPATCH], F32, tag=f"u{j}")
                                nc.vector.tensor_scalar_mul(
                                    u[:], sigb[:, :, j, 0],
                                    wq_h[:, k, 2 * j : 2 * j + 1],
                                )
                                m = wk.tile([128, NPATCH], F32, tag=f"m{j}")
                                nc.vector.scalar_tensor_tensor(
                                    m[:], sigb[:, :, j, 1],
                                    wq_h[:, k, 2 * j + 1 : 2 * j + 2],
                                    u[:], MUL, ADD,
                                )
                                m2 = wk.tile([128, NPATCH], F32, tag=f"m2{j}")
                                nc.vector.tensor_scalar(
                                    m2[:], m[:], wq_h[:, k, 4 + j : 5 + j],
                                    None, ADD,
                                )
                                mj.append(m2)
                            p0 = wk.tile([128, NPATCH], F32, tag="p0")
                            nc.vector.tensor_mul(
                                p0[:], at[:, k, :, 0], mj[0][:]
                            )
                            p1 = wk.tile([128, NPATCH], F32, tag="p1")
                            nc.vector.tensor_mul(
                                p1[:], at[:, k, :, 1], mj[1][:]
                            )
                            nc.vector.tensor_add(
                                acm[:, k, 1:NOUT], p0[:], p1[:]
                            )

                # -- pass B: finals + stores for this group --
                pass_b(bs, acms)

    nc.compile()
    return nc


def prepare(x, edge, W_lin, W_out_cls, weights):
    """Host-side prep shared by kernel() and the timing harness: returns
    (w_scalars, in_maps)."""
    x = np.ascontiguousarray(x, dtype=np.float32)
    edge = np.ascontiguousarray(edge, dtype=np.float32)

    # x channel-major: [B, 128, 3, 784]; free order = (chunk, patch, quad),
    # patch = (i, j) row-major, quad = 2*r + cc
    nodes = x[:, 1:, :].reshape(B, HP, 2, HP, 2, C)
    xcm = (
        nodes.transpose(0, 5, 1, 3, 2, 4)        # [b, c, i, j, r, cc]
        .reshape(B, NCH, 128, HW)
        .transpose(0, 2, 1, 3)                   # [b, p, chunk, n]
        .astype(np.float16)
    )
    # edge channel-major fp8 in 4 token-quarters (DMA-CCE accumulated on
    # device), pre-scaled by 784/785 so the device can use a single 1/784
    # sigmoid scale for both the edge and node means
    ecm = np.zeros((B, C, NE), dtype=np.float32)
    ecm[:, :, 0:N] = edge.transpose(0, 2, 1) * (float(HW) / N)
    ecm8 = (
        ecm.reshape(B, NCH, 128, NE)
        .transpose(0, 2, 1, 3)                   # [b, p, chunk, t]
        .astype(ml_dtypes.float8_e4m3)
    )

    wlt = np.asarray(W_lin, dtype=np.float32).T          # [c, c']
    wlt_cm = (
        wlt.reshape(NCH, 128, C).transpose(1, 0, 2).astype(np.float16)
    )
    wct = np.asarray(W_out_cls, dtype=np.float32).T      # [c, co]
    wct_cm = (
        wct.reshape(NCH, 128, CO).transpose(1, 0, 2).astype(np.float16)
    )
    w = np.asarray(weights, dtype=np.float32)
    c_uniform = bool(np.all(w == w[0:1]))
    w_scalars = tuple(float(v) for v in w[0].reshape(4)) if c_uniform else None

    in_maps = []
    for core in range(NCORES):
        sl = slice(core * NB, (core + 1) * NB)
        cls_cm = (
            x[sl, 0, :].T.reshape(NCH, 128, NB)
            .transpose(1, 0, 2)
            .astype(np.float16)
        )
        m = {
            "x": xcm[sl], "edge": ecm8[sl], "wlt": wlt_cm, "wct": wct_cm,
            "cls_cm": np.ascontiguousarray(cls_cm),
        }
        if w_scalars is None:
            wq4 = w.reshape(C, 4)
            wq = np.concatenate(
                [wq4, wq4[:, 0:2].sum(1, keepdims=True),
                 wq4[:, 2:4].sum(1, keepdims=True)], axis=1
            )                                            # [C, 6]
            wq = wq.reshape(NCH, 128, 6).transpose(1, 0, 2)
            m["wq_cm"] = np.ascontiguousarray(wq, dtype=np.float32)
        in_maps.append(m)
    return w_scalars, in_maps


def kernel(x, edge, W_lin, W_out_cls, weights):
    w_scalars, in_maps = prepare(x, edge, W_lin, W_out_cls, weights)
    nc = build_program(w_scalars)
    res = run_bass_kernel_spmd(nc, in_maps, list(range(NCORES)))
    out = np.concatenate(
        [r["out"].astype(np.float32) for r in res.results], axis=0
    )
    return out


# revision 55
# speedup vs baseline: 1.0067x; 1.0067x over previous
"""Trainium2 Bass kernel for nn_Pooling_block (B=128, N=785, C=384, pp=2).

Pure data-parallel over batch: 16 batches per core x 8 NeuronCores.
~115 us HW exec (baseline was 230 us); rel err ~1.7e-3 (gate 2e-2).

Final design ("c-major", fp16 x / fp8 edge / fp16 out, balanced SDMA):
  - x in float16 channel-major [B, 128, 3, 784]; edge in float8-e4m3
    c-major [B, 128, 3, 788] (3 zero-pad tokens); output stored fp16 and
    upcast on host.  Per-core HBM traffic: 15.4 MB reads + 4.8 MB writes
    (vs 56 MB all-f32).  fp8 only feeds a mean -> sigmoid and is
    accumulated in f32; fp16 everywhere else keeps the error ~1.7e-3.
  - all compute is 128-partition-wide c-major (no PE transposes, no
    single-lane [1, N] ops):
      * pair sums + node sums: 3 DVE stt ops with accum_out
      * edge sums: 3 DVE stt token-half folds with accum_out
      * one sigmoid per batch covers both means ([128, 2, 3] tile; edge
        host-pre-scaled by 784/785 so a single 1/784 scale fits both)
      * group ci: 9 matmuls (W_lin.T blocks stationary)
      * scores: 6 matmuls with the ci column stride-0-BROADCAST to
        [128,128] as the stationary operand -> scores land replicated on
        all 128 partitions, keeping sigmoid/t01/pooled full-width
      * t01: ONE stt op: (sig0 + 2) + sig1 -> fp16
      * pooled: fp16 mul (t01 stride-0 broadcast across chunks) on DVE +
        add on gpsimd (DVE for tail batches) writing fp16 into acm
  - finals: 12 fp16 matmuls vs W_out_cls.T chunks; PSUM->SBUF on ACT.
  - stores: [128, 768] rows via sync HWDGE (even p//8 engine split); the
    69-row tail via gpsimd SWDGE, whose swizzle spreads <128-partition
    transfers across all 16 SDMA engines (HWDGE puts 69 rows on just 3
    engines, which was the v4 bottleneck); last batch tail via ACT HWDGE
    (DMA is idle then, latency wins).
  - failed experiments kept out: DMA-CCE accumulate for edge halves (the
    single SWDGE queue is strictly FIFO, so accum chains head-of-line
    block behind stores), >2-deep accum chains, s-add on gpsimd.
"""
import os
import sys

sys.path.insert(0, "/opt/trn_rl_repo")

import numpy as np
import ml_dtypes

import concourse.bass as bass
import concourse.tile as tile
from concourse import bacc, mybir
from concourse.bass_utils import run_bass_kernel_spmd

B, N, C = 128, 785, 384
HW = N - 1          # 784
H = 28              # grid side
HP = 14             # pooled grid side
NPATCH = HP * HP    # 196
NB = 16             # batches per core
NCORES = 8
NOUT = 1 + NPATCH   # 197
CO = 2 * C          # 768
NCH = 3             # channel chunks of 128
GRP = 4             # max batches per group
NE = 788            # padded edge tokens (2 * 394)
EQ = NE // 2        # 394: tokens per CCE-accumulated half
EH = EQ // 2        # 197: tokens per DVE fold half

F32 = mybir.dt.float32
F16 = mybir.dt.float16
F8 = mybir.dt.float8e4
ADD = mybir.AluOpType.add
MUL = mybir.AluOpType.mult
SIGMOID = mybir.ActivationFunctionType.Sigmoid
COPY = mybir.ActivationFunctionType.Copy
AXC = mybir.AxisListType.X


def build_program(w_scalars):
    """Build the per-core SPMD program. w_scalars = (w00, w01, w10, w11) when
    the per-patch weights are channel-uniform, else None (general path)."""
    nc = bacc.Bacc(None, target_bir_lowering=False, debug=False)

    x_d = nc.declare_dram_parameter("x", [NB, 128, NCH, HW], F16, isOutput=False)
    e_d = nc.declare_dram_parameter("edge", [NB, 128, NCH, NE], F8, isOutput=False)
    wlt_d = nc.declare_dram_parameter("wlt", [128, NCH, C], F16, isOutput=False)
    wct_d = nc.declare_dram_parameter("wct", [128, NCH, CO], F16, isOutput=False)
    clsc_d = nc.declare_dram_parameter("cls_cm", [128, NCH, NB], F16, isOutput=False)
    general_w = w_scalars is None
    if general_w:
        # cols 0-3: w[c, q]; cols 4-5: w[c,2j] + w[c,2j+1]
        wq_d = nc.declare_dram_parameter("wq_cm", [128, NCH, 6], F32, isOutput=False)
    # output leaves the device in fp16 (host upcasts): halves store traffic;
    # quantization (~5e-4 of abs-max) is far below the pipeline's ~1.7e-3
    out_d = nc.declare_dram_parameter("out", [NB, NOUT, CO], F16, isOutput=True)

    uniform_w = (not general_w) and len(set(w_scalars)) == 1

    with tile.TileContext(nc) as tc:
        with (
            tc.tile_pool(name="const", bufs=1) as cpool,
            tc.tile_pool(name="gx", bufs=8) as gxp,
            tc.tile_pool(name="ed", bufs=8) as edp,
            tc.tile_pool(name="apool", bufs=8) as ap,
            tc.tile_pool(name="work", bufs=4) as wk,
            tc.tile_pool(name="small", bufs=4) as sm,
            tc.tile_pool(name="sgp", bufs=2) as sgp,
            tc.tile_pool(name="acm", bufs=8) as acmp,
            tc.tile_pool(name="ost", bufs=6) as ostp,
            tc.tile_pool(name="psS", bufs=3, space="PSUM") as psS,
            tc.tile_pool(name="psC", bufs=2, space="PSUM") as psC,
            tc.tile_pool(name="psF", bufs=3, space="PSUM") as psF,
        ):
            # all constant loads queue AFTER batch 0's data (the ramp's
            # critical path); wlt still lands before the first group-ci
            # matmuls need it
            wlt_h = cpool.tile([128, NCH, C], F16)
            cls_h = cpool.tile([128, NCH, NB], F16)
            wct_h = cpool.tile([128, NCH, CO], F16)
            if general_w:
                wq_h = cpool.tile([128, NCH, 6], F32)

            # size-1 head groups fill the pipeline fastest (shortest chain
            # to the first scores); size-1 tail groups drain it fastest.
            # [2,4,4,4,2] was re-measured ~2.5us slower than this split.
            group_list = [range(0, 1), range(1, 2), range(2, 6), range(6, 10),
                          range(10, 14), range(14, 15), range(15, 16)]
            first = True

            def pass_b(bs, acms):
                # finals + stores for one group
                for b in bs:
                    acm = acms[b]
                    for r0, rn in ((0, 128), (128, 69)):
                        stile = ostp.tile([128, CO], F16, tag="ost",
                                          name="stile")
                        for nh in range(2):
                            fo = psF.tile([128, C], F32, tag="fo", name="fo")
                            for cch in range(NCH):
                                nc.tensor.matmul(
                                    fo[0:rn, :],
                                    acm[:, cch, r0 : r0 + rn],
                                    wct_h[:, cch, C * nh : C * (nh + 1)],
                                    start=(cch == 0), stop=(cch == 2),
                                )
                            nc.scalar.copy(
                                stile[0:rn, C * nh : C * (nh + 1)], fo[0:rn, :]
                            )
                        if rn == 128:
                            # 128-row store: HWDGE splits evenly (p//8);
                            # issued from sync to keep ACT free for copies
                            nc.sync.dma_start(
                                out_d[b, r0 : r0 + rn, :], stile[0:rn, :]
                            )
                        elif b == NB - 1:
                            # tail batch: DMA system is idle by now, so the
                            # low-latency HWDGE path beats SWDGE's spread
                            nc.scalar.dma_start(
                                out_d[b, r0 : r0 + rn, :], stile[0:rn, :]
                            )
                        else:
                            # <128-row store: SWDGE swizzle spreads it across
                            # all 16 SDMA engines (HWDGE would use only 3)
                            nc.gpsimd.dma_start(
                                out_d[b, r0 : r0 + rn, :], stile[0:rn, :]
                            )

            # NOTE on pass-B ordering: issuing finals one group late (to
            # keep tail scores matmuls from queueing behind a 48-matmul
            # finals wall in the PE stream - a measured 7.2us DVE stall)
            # was tried twice.  Deferring every group measured +25us (acm
            # buffer rotation couples pass A to the previous finals), and
            # deferring only the tail groups produced a NONDETERMINISTIC
            # wrong result on hardware with no speedup.  Keep pass B
            # strictly inline per group.
            for gi, bs in enumerate(group_list):
                glen = len(bs)
                x_t, a_t = {}, {}

                # s columns for the group, c-major: [128, chunk, batch]
                sG = sgp.tile([128, NCH, GRP], F16, tag="sG")

                # -- sub-loop 1: loads + per-batch token sums -> s columns --
                for b in bs:
                    gb = b - bs[0]
                    xb = gxp.tile([128, NCH, NPATCH, 4], F16, tag="xb")
                    nc.sync.dma_start(
                        xb[:],
                        x_d[b].rearrange("p k (n q) -> p k n q", q=4),
                    )
                    x_t[b] = xb
                    eb = edp.tile([128, NCH, NE], F8, tag="eb")
                    nc.sync.dma_start(eb[:], e_d[b])

                    if first:
                        nc.sync.dma_start(wlt_h[:], wlt_d[:])
                        nc.sync.dma_start(cls_h[:], clsc_d[:])
                        nc.sync.dma_start(wct_h[:], wct_d[:])
                        if general_w:
                            nc.sync.dma_start(wq_h[:], wq_d[:])
                        first = False

                    # esns[:, 0, k] = edge sums, esns[:, 1, k] = node sums
                    esns = sm.tile([128, 2, NCH], F32, tag="esns")

                    # pair sums + node sums in one pass (3 stt ops w/ accum)
                    at = ap.tile([128, NCH, NPATCH, 2], F16, tag="at")
                    for k in range(NCH):
                        nc.vector.scalar_tensor_tensor(
                            at[:, k], xb[:, k, :, 0:2], 1.0, xb[:, k, :, 2:4],
                            MUL, ADD, accum_out=esns[:, 1, k : k + 1],
                        )
                    a_t[b] = at

                    # edge sums: fold the two token-halves, f32 accumulate.
                    # (Reducing a chunk on ACT via Copy+accum_out was tried
                    # and regressed ~20us: the load-gated op head-of-line
                    # blocks ACT's in-order stream of sigmoids and copies.)
                    ef = wk.tile([128, NCH, EQ], F16, tag="ef")
                    for k in range(NCH):
                        nc.vector.scalar_tensor_tensor(
                            ef[:, k], eb[:, k, 0:EQ], 1.0, eb[:, k, EQ:NE],
                            MUL, ADD, accum_out=esns[:, 0, k : k + 1],
                        )

                    # one sigmoid for both means (edge host-pre-scaled)
                    sg2 = sm.tile([128, 2, NCH], F32, tag="sg2")
                    nc.scalar.activation(
                        sg2[:], esns[:], SIGMOID, scale=1.0 / HW
                    )
                    nc.vector.tensor_add(
                        sG[:, :, gb : gb + 1],
                        sg2[:, 0:1, :].rearrange("p u k -> p k u"),
                        sg2[:, 1:2, :].rearrange("p u k -> p k u"),
                    )

                # -- group ci: 9 matmuls, all c-major, no transposes --
                ciG_ps = psC.tile([128, NCH, GRP], F32, tag="ciG")
                for m in range(NCH):
                    for k in range(NCH):
                        nc.tensor.matmul(
                            ciG_ps[:, m, 0:glen],
                            wlt_h[:, k, 128 * m : 128 * (m + 1)],
                            sG[:, k, 0:glen],
                            start=(k == 0), stop=(k == 2),
                        )
                ciG = sm.tile([128, NCH, GRP], F16, tag="ciG_sb")
                nc.scalar.copy(ciG[:], ciG_ps[:])

                # -- pass A: scores -> t01 -> pooled -> acm per batch --
                acms = {}
                for b in bs:
                    gb = b - bs[0]
                    xb, at = x_t[b], a_t[b]

                    # scores, replicated on all 128 partitions via a
                    # stride-0 broadcast of the ci column as lhsT
                    sigb = sm.tile([128, NPATCH, 2, 2], F16, tag="sigb")
                    for h in range(2):
                        sc_ps = psS.tile([128, 392], F32, tag="sc")
                        for k in range(NCH):
                            nc.tensor.matmul(
                                sc_ps[:],
                                ciG[:, k, gb : gb + 1].to_broadcast([128, 128]),
                                xb[:, k, 98 * h : 98 * (h + 1), :].rearrange(
                                    "p n q -> p (n q)"
                                ),
                                start=(k == 0), stop=(k == 2),
                            )
                        nc.scalar.activation(
                            sigb[:, 98 * h : 98 * (h + 1)].rearrange(
                                "p n j r -> p (n j r)"
                            ),
                            sc_ps[:], SIGMOID,
                        )

                    acm = acmp.tile([128, NCH, NOUT], F16, tag="acm")
                    nc.gpsimd.tensor_copy(acm[:, :, 0:1], cls_h[:, :, b : b + 1])
                    acms[b] = acm

                    if not general_w:
                        # t01h has a singleton chunk dim for stride-0 bcast
                        t01h = wk.tile([128, 1, NPATCH, 2], F16, tag="t01h")
                        if uniform_w and w_scalars[0] == 1.0:
                            # t01[p,j] = (sig(4p+2j) + 2) + sig(4p+2j+1)
                            nc.vector.scalar_tensor_tensor(
                                t01h[:, 0], sigb[:, :, :, 0:1], 2.0,
                                sigb[:, :, :, 1:2], ADD, ADD,
                            )
                        elif uniform_w:
                            w00 = w_scalars[0]
                            t01f = wk.tile([128, NPATCH, 2], F32, tag="t01f")
                            nc.vector.scalar_tensor_tensor(
                                t01f[:], sigb[:, :, :, 0:1], 2.0,
                                sigb[:, :, :, 1:2], ADD, ADD,
                            )
                            nc.vector.tensor_scalar(
                                t01h[:, 0], t01f[:], w00, None, MUL
                            )
                        else:
                            wv = w_scalars
                            for j in range(2):
                                vj = wk.tile([128, NPATCH], F32, tag=f"vj{j}")
                                nc.vector.tensor_scalar(
                                    vj[:], sigb[:, :, j, 1],
                                    float(wv[2 * j + 1]),
                                    float(wv[2 * j] + wv[2 * j + 1]),
                                    MUL, ADD,
                                )
                                nc.vector.scalar_tensor_tensor(
                                    t01h[:, 0, :, j], sigb[:, :, j, 0],
                                    float(wv[2 * j]), vj[:], MUL, ADD,
                                )
                        prod = wk.tile([128, NCH, NPATCH, 2], F16, tag="prod")
                        nc.vector.tensor_mul(
                            prod[:], at[:],
                            t01h[:].to_broadcast([128, NCH, NPATCH, 2]),
                        )
                        # acm add off DVE (the global bottleneck) except for
                        # the tail batches, where gpsimd's slow op + the
                        # cross-engine hop would sit on the critical path
                        acm_eng = nc.vector if b >= NB - 2 else nc.gpsimd
                        acm_eng.tensor_add(
                            acm[:, :, 1:NOUT].rearrange(
                                "p k (n u) -> p k n u", u=1
                            ),
                            prod[:, :, :, 0:1], prod[:, :, :, 1:2],
                        )
                    else:
                        # fully general per-channel weights: m_j[c, p] =
                        # w[c,2j]*sig_j0 + w[c,2j+1]*sig_j1 + wsum_j[c]
                        for k in range(NCH):
                            mj = []
                            for j in range(2):
                                u = wk.tile([128, NPATCH], F32, tag=f"u{j}")
                                nc.vector.tensor_scalar_mul(
                                    u[:], sigb[:, :, j, 0],
                                    wq_h[:, k, 2 * j : 2 * j + 1],
                                )
                                m = wk.tile([128, NPATCH], F32, tag=f"m{j}")
                                nc.vector.scalar_tensor_tensor(
                                    m[:], sigb[:, :, j, 1],
                                    wq_h[:, k, 2 * j + 1 : 2 * j + 2],
                                    u[:], MUL, ADD,
                                )
                                m2 = wk.tile([128, NPATCH], F32, tag=f"m2{j}")
                                nc.vector.tensor_scalar(
                                    m2[:], m[:], wq_h[:, k, 4 + j : 5 + j],
                                    None, ADD,
                                )
                                mj.append(m2)
                            p0 = wk.tile([128, NPATCH], F32, tag="p0")
                            nc.vector.tensor_mul(
                                p0[:], at[:, k, :, 0], mj[0][:]
                            )
                            p1 = wk.tile([128, NPATCH], F32, tag="p1")
                            nc.vector.tensor_mul(
                                p1[:], at[:, k, :, 1], mj[1][:]
                            )
                            nc.vector.tensor_add(
                                acm[:, k, 1:NOUT], p0[:], p1[:]
                            )

                # -- pass B: finals + stores for this group --
                pass_b(bs, acms)

    nc.compile()
    return nc


def prepare(x, edge, W_lin, W_out_cls, weights):
    """Host-side prep shared by kernel() and the timing harness: returns
    (w_scalars, in_maps)."""
    x = np.ascontiguousarray(x, dtype=np.float32)
    edge = np.ascontiguousarray(edge, dtype=np.float32)

    # x channel-major: [B, 128, 3, 784]; free order = (chunk, patch, quad),
    # patch = (i, j) row-major, quad = 2*r + cc
    nodes = x[:, 1:, :].reshape(B, HP, 2, HP, 2, C)
    xcm = (
        nodes.transpose(0, 5, 1, 3, 2, 4)        # [b, c, i, j, r, cc]
        .reshape(B, NCH, 128, HW)
        .transpose(0, 2, 1, 3)                   # [b, p, chunk, n]
        .astype(np.float16)
    )
    # edge channel-major fp8 in 4 token-quarters (DMA-CCE accumulated on
    # device), pre-scaled by 784/785 so the device can use a single 1/784
    # sigmoid scale for both the edge and node means
    ecm = np.zeros((B, C, NE), dtype=np.float32)
    ecm[:, :, 0:N] = edge.transpose(0, 2, 1) * (float(HW) / N)
    ecm8 = (
        ecm.reshape(B, NCH, 128, NE)
        .transpose(0, 2, 1, 3)                   # [b, p, chunk, t]
        .astype(ml_dtypes.float8_e4m3)
    )

    wlt = np.asarray(W_lin, dtype=np.float32).T          # [c, c']
    wlt_cm = (
        wlt.reshape(NCH, 128, C).transpose(1, 0, 2).astype(np.float16)
    )
    wct = np.asarray(W_out_cls, dtype=np.float32).T      # [c, co]
    wct_cm = (
        wct.reshape(NCH, 128, CO).transpose(1, 0, 2).astype(np.float16)
    )
    w = np.asarray(weights, dtype=np.float32)
    c_uniform = bool(np.all(w == w[0:1]))
    w_scalars = tuple(float(v) for v in w[0].reshape(4)) if c_uniform else None

    in_maps = []
    for core in range(NCORES):
        sl = slice(core * NB, (core + 1) * NB)
        cls_cm = (
            x[sl, 0, :].T.reshape(NCH, 128, NB)
            .transpose(1, 0, 2)
            .astype(np.float16)
        )
        m = {
            "x": xcm[sl], "edge": ecm8[sl], "wlt": wlt_cm, "wct": wct_cm,
            "cls_cm": np.ascontiguousarray(cls_cm),
        }
        if w_scalars is None:
            wq4 = w.reshape(C, 4)
            wq = np.concatenate(
                [wq4, wq4[:, 0:2].sum(1, keepdims=True),
                 wq4[:, 2:4].sum(1, keepdims=True)], axis=1
            )                                            # [C, 6]
            wq = wq.reshape(NCH, 128, 6).transpose(1, 0, 2)
            m["wq_cm"] = np.ascontiguousarray(wq, dtype=np.float32)
        in_maps.append(m)
    return w_scalars, in_maps


def kernel(x, edge, W_lin, W_out_cls, weights):
    w_scalars, in_maps = prepare(x, edge, W_lin, W_out_cls, weights)
    nc = build_program(w_scalars)
    res = run_bass_kernel_spmd(nc, in_maps, list(range(NCORES)))
    out = np.concatenate(
        [r["out"].astype(np.float32) for r in res.results], axis=0
    )
    return out


# revision 56
# speedup vs baseline: 1.0288x; 1.0220x over previous
"""Trainium2 Bass kernel for nn_Pooling_block (B=128, N=785, C=384, pp=2).

Pure data-parallel over batch: 16 batches per core x 8 NeuronCores.
~115 us HW exec (baseline was 230 us); rel err ~1.7e-3 (gate 2e-2).

Final design ("c-major", fp16 x / fp8 edge / fp16 out, balanced SDMA):
  - x in float16 channel-major [B, 128, 3, 784]; edge in float8-e4m3
    c-major [B, 128, 3, 788] (3 zero-pad tokens); output stored fp16 and
    upcast on host.  Per-core HBM traffic: 15.4 MB reads + 4.8 MB writes
    (vs 56 MB all-f32).  fp8 only feeds a mean -> sigmoid and is
    accumulated in f32; fp16 everywhere else keeps the error ~1.7e-3.
  - all compute is 128-partition-wide c-major (no PE transposes, no
    single-lane [1, N] ops):
      * pair sums + node sums: 3 DVE stt ops with accum_out
      * edge sums: 3 DVE stt token-half folds with accum_out
      * one sigmoid per batch covers both means ([128, 2, 3] tile; edge
        host-pre-scaled by 784/785 so a single 1/784 scale fits both)
      * group ci: 9 matmuls (W_lin.T blocks stationary)
      * scores: 6 matmuls with the ci column stride-0-BROADCAST to
        [128,128] as the stationary operand -> scores land replicated on
        all 128 partitions, keeping sigmoid/t01/pooled full-width
      * t01: ONE stt op: (sig0 + 2) + sig1 -> fp16
      * pooled: fp16 mul (t01 stride-0 broadcast across chunks) on DVE +
        add on gpsimd (DVE for tail batches) writing fp16 into acm
  - finals: 12 fp16 matmuls vs W_out_cls.T chunks; PSUM->SBUF on ACT.
  - stores: [128, 768] rows via sync HWDGE (even p//8 engine split); the
    69-row tail via gpsimd SWDGE, whose swizzle spreads <128-partition
    transfers across all 16 SDMA engines (HWDGE puts 69 rows on just 3
    engines, which was the v4 bottleneck); last batch tail via ACT HWDGE
    (DMA is idle then, latency wins).
  - failed experiments kept out: DMA-CCE accumulate for edge halves (the
    single SWDGE queue is strictly FIFO, so accum chains head-of-line
    block behind stores), >2-deep accum chains, s-add on gpsimd.
"""
import os
import sys

sys.path.insert(0, "/opt/trn_rl_repo")

import numpy as np
import ml_dtypes

import concourse.bass as bass
import concourse.tile as tile
from concourse import bacc, mybir
from concourse.bass_utils import run_bass_kernel_spmd

B, N, C = 128, 785, 384
HW = N - 1          # 784
H = 28              # grid side
HP = 14             # pooled grid side
NPATCH = HP * HP    # 196
NB = 16             # batches per core
NCORES = 8
NOUT = 1 + NPATCH   # 197
CO = 2 * C          # 768
NCH = 3             # channel chunks of 128
GRP = 4             # max batches per group
NE = 788            # padded edge tokens (2 * 394)
EQ = NE // 2        # 394: tokens per CCE-accumulated half
EH = EQ // 2        # 197: tokens per DVE fold half

F32 = mybir.dt.float32
F16 = mybir.dt.float16
F8 = mybir.dt.float8e4
ADD = mybir.AluOpType.add
MUL = mybir.AluOpType.mult
SIGMOID = mybir.ActivationFunctionType.Sigmoid
COPY = mybir.ActivationFunctionType.Copy
AXC = mybir.AxisListType.X


def build_program(w_scalars):
    """Build the per-core SPMD program. w_scalars = (w00, w01, w10, w11) when
    the per-patch weights are channel-uniform, else None (general path)."""
    nc = bacc.Bacc(None, target_bir_lowering=False, debug=False)

    x_d = nc.declare_dram_parameter("x", [NB, 128, NCH, HW], F16, isOutput=False)
    e_d = nc.declare_dram_parameter("edge", [NB, 128, NCH, NE], F8, isOutput=False)
    wlt_d = nc.declare_dram_parameter("wlt", [128, NCH, C], F16, isOutput=False)
    wct_d = nc.declare_dram_parameter("wct", [128, NCH, CO], F16, isOutput=False)
    clsc_d = nc.declare_dram_parameter("cls_cm", [128, NCH, NB], F16, isOutput=False)
    general_w = w_scalars is None
    if general_w:
        # cols 0-3: w[c, q]; cols 4-5: w[c,2j] + w[c,2j+1]
        wq_d = nc.declare_dram_parameter("wq_cm", [128, NCH, 6], F32, isOutput=False)
    # output leaves the device in fp16 (host upcasts): halves store traffic;
    # quantization (~5e-4 of abs-max) is far below the pipeline's ~1.7e-3
    out_d = nc.declare_dram_parameter("out", [NB, NOUT, CO], F16, isOutput=True)

    uniform_w = (not general_w) and len(set(w_scalars)) == 1

    with tile.TileContext(nc) as tc:
        with (
            tc.tile_pool(name="const", bufs=1) as cpool,
            tc.tile_pool(name="gx", bufs=8) as gxp,
            tc.tile_pool(name="ed", bufs=8) as edp,
            tc.tile_pool(name="apool", bufs=8) as ap,
            # work bufs=8: prod(b) is consumed by gpsimd's acm-add, which
            # sits late in gpsimd's stream (behind stores) - at bufs=4 the
            # DVE's prod(b+4) write stalls on that slow consumer
            tc.tile_pool(name="work", bufs=8) as wk,
            tc.tile_pool(name="small", bufs=4) as sm,
            tc.tile_pool(name="sgp", bufs=2) as sgp,
            tc.tile_pool(name="acm", bufs=10) as acmp,
            tc.tile_pool(name="ost", bufs=8) as ostp,
            tc.tile_pool(name="psS", bufs=3, space="PSUM") as psS,
            tc.tile_pool(name="psC", bufs=2, space="PSUM") as psC,
            tc.tile_pool(name="psF", bufs=3, space="PSUM") as psF,
        ):
            # all constant loads queue AFTER batch 0's data (the ramp's
            # critical path); wlt still lands before the first group-ci
            # matmuls need it
            wlt_h = cpool.tile([128, NCH, C], F16)
            cls_h = cpool.tile([128, NCH, NB], F16)
            wct_h = cpool.tile([128, NCH, CO], F16)
            if general_w:
                wq_h = cpool.tile([128, NCH, 6], F32)

            # size-1 head groups fill the pipeline fastest (shortest chain
            # to the first scores); size-1 tail groups drain it fastest.
            # [2,4,4,4,2] was re-measured ~2.5us slower than this split.
            group_list = [range(0, 1), range(1, 2), range(2, 6), range(6, 10),
                          range(10, 14), range(14, 15), range(15, 16)]
            first = True

            def pass_b(bs, acms):
                # finals + stores for one group
                for b in bs:
                    acm = acms[b]
                    for r0, rn in ((0, 128), (128, 69)):
                        stile = ostp.tile([128, CO], F16, tag="ost",
                                          name="stile")
                        for nh in range(2):
                            fo = psF.tile([128, C], F32, tag="fo", name="fo")
                            for cch in range(NCH):
                                nc.tensor.matmul(
                                    fo[0:rn, :],
                                    acm[:, cch, r0 : r0 + rn],
                                    wct_h[:, cch, C * nh : C * (nh + 1)],
                                    start=(cch == 0), stop=(cch == 2),
                                )
                            nc.scalar.copy(
                                stile[0:rn, C * nh : C * (nh + 1)], fo[0:rn, :]
                            )
                        if rn == 128:
                            # 128-row store: HWDGE splits evenly (p//8);
                            # issued from sync to keep ACT free for copies
                            nc.sync.dma_start(
                                out_d[b, r0 : r0 + rn, :], stile[0:rn, :]
                            )
                        elif b == NB - 1:
                            # tail batch: DMA system is idle by now, so the
                            # low-latency HWDGE path beats SWDGE's spread
                            nc.scalar.dma_start(
                                out_d[b, r0 : r0 + rn, :], stile[0:rn, :]
                            )
                        else:
                            # <128-row store: SWDGE swizzle spreads it across
                            # all 16 SDMA engines (HWDGE would use only 3)
                            nc.gpsimd.dma_start(
                                out_d[b, r0 : r0 + rn, :], stile[0:rn, :]
                            )

            # NOTE on pass-B ordering: issuing finals one group late (to
            # keep tail scores matmuls from queueing behind a 48-matmul
            # finals wall in the PE stream - a measured 7.2us DVE stall)
            # was tried twice.  Deferring every group measured +25us (acm
            # buffer rotation couples pass A to the previous finals), and
            # deferring only the tail groups produced a NONDETERMINISTIC
            # wrong result on hardware with no speedup.  Keep pass B
            # strictly inline per group.
            for gi, bs in enumerate(group_list):
                glen = len(bs)
                x_t, a_t = {}, {}

                # s columns for the group, c-major: [128, chunk, batch]
                sG = sgp.tile([128, NCH, GRP], F16, tag="sG")

                # -- sub-loop 1: loads + per-batch token sums -> s columns --
                for b in bs:
                    gb = b - bs[0]
                    xb = gxp.tile([128, NCH, NPATCH, 4], F16, tag="xb")
                    nc.sync.dma_start(
                        xb[:],
                        x_d[b].rearrange("p k (n q) -> p k n q", q=4),
                    )
                    x_t[b] = xb
                    eb = edp.tile([128, NCH, NE], F8, tag="eb")
                    nc.sync.dma_start(eb[:], e_d[b])

                    if first:
                        nc.sync.dma_start(wlt_h[:], wlt_d[:])
                        nc.sync.dma_start(cls_h[:], clsc_d[:])
                        nc.sync.dma_start(wct_h[:], wct_d[:])
                        if general_w:
                            nc.sync.dma_start(wq_h[:], wq_d[:])
                        first = False

                    # esns[:, 0, k] = edge sums, esns[:, 1, k] = node sums
                    esns = sm.tile([128, 2, NCH], F32, tag="esns")

                    # pair sums + node sums in one pass (3 stt ops w/ accum)
                    at = ap.tile([128, NCH, NPATCH, 2], F16, tag="at")
                    for k in range(NCH):
                        nc.vector.scalar_tensor_tensor(
                            at[:, k], xb[:, k, :, 0:2], 1.0, xb[:, k, :, 2:4],
                            MUL, ADD, accum_out=esns[:, 1, k : k + 1],
                        )
                    a_t[b] = at

                    # edge sums: fold the two token-halves, f32 accumulate.
                    # (Reducing a chunk on ACT via Copy+accum_out was tried
                    # and regressed ~20us: the load-gated op head-of-line
                    # blocks ACT's in-order stream of sigmoids and copies.)
                    ef = wk.tile([128, NCH, EQ], F16, tag="ef")
                    for k in range(NCH):
                        nc.vector.scalar_tensor_tensor(
                            ef[:, k], eb[:, k, 0:EQ], 1.0, eb[:, k, EQ:NE],
                            MUL, ADD, accum_out=esns[:, 0, k : k + 1],
                        )

                    # one sigmoid for both means (edge host-pre-scaled)
                    sg2 = sm.tile([128, 2, NCH], F32, tag="sg2")
                    nc.scalar.activation(
                        sg2[:], esns[:], SIGMOID, scale=1.0 / HW
                    )
                    nc.vector.tensor_add(
                        sG[:, :, gb : gb + 1],
                        sg2[:, 0:1, :].rearrange("p u k -> p k u"),
                        sg2[:, 1:2, :].rearrange("p u k -> p k u"),
                    )

                # -- group ci: 9 matmuls, all c-major, no transposes --
                ciG_ps = psC.tile([128, NCH, GRP], F32, tag="ciG")
                for m in range(NCH):
                    for k in range(NCH):
                        nc.tensor.matmul(
                            ciG_ps[:, m, 0:glen],
                            wlt_h[:, k, 128 * m : 128 * (m + 1)],
                            sG[:, k, 0:glen],
                            start=(k == 0), stop=(k == 2),
                        )
                ciG = sm.tile([128, NCH, GRP], F16, tag="ciG_sb")
                nc.scalar.copy(ciG[:], ciG_ps[:])

                # -- pass A: scores -> t01 -> pooled -> acm per batch --
                acms = {}
                for b in bs:
                    gb = b - bs[0]
                    xb, at = x_t[b], a_t[b]

                    # scores, replicated on all 128 partitions via a
                    # stride-0 broadcast of the ci column as lhsT
                    sigb = sm.tile([128, NPATCH, 2, 2], F16, tag="sigb")
                    for h in range(2):
                        sc_ps = psS.tile([128, 392], F32, tag="sc")
                        for k in range(NCH):
                            nc.tensor.matmul(
                                sc_ps[:],
                                ciG[:, k, gb : gb + 1].to_broadcast([128, 128]),
                                xb[:, k, 98 * h : 98 * (h + 1), :].rearrange(
                                    "p n q -> p (n q)"
                                ),
                                start=(k == 0), stop=(k == 2),
                            )
                        nc.scalar.activation(
                            sigb[:, 98 * h : 98 * (h + 1)].rearrange(
                                "p n j r -> p (n j r)"
                            ),
                            sc_ps[:], SIGMOID,
                        )

                    acm = acmp.tile([128, NCH, NOUT], F16, tag="acm")
                    nc.gpsimd.tensor_copy(acm[:, :, 0:1], cls_h[:, :, b : b + 1])
                    acms[b] = acm

                    if not general_w:
                        # t01h has a singleton chunk dim for stride-0 bcast
                        t01h = wk.tile([128, 1, NPATCH, 2], F16, tag="t01h")
                        if uniform_w and w_scalars[0] == 1.0:
                            # t01[p,j] = (sig(4p+2j) + 2) + sig(4p+2j+1)
                            nc.vector.scalar_tensor_tensor(
                                t01h[:, 0], sigb[:, :, :, 0:1], 2.0,
                                sigb[:, :, :, 1:2], ADD, ADD,
                            )
                        elif uniform_w:
                            w00 = w_scalars[0]
                            t01f = wk.tile([128, NPATCH, 2], F32, tag="t01f")
                            nc.vector.scalar_tensor_tensor(
                                t01f[:], sigb[:, :, :, 0:1], 2.0,
                                sigb[:, :, :, 1:2], ADD, ADD,
                            )
                            nc.vector.tensor_scalar(
                                t01h[:, 0], t01f[:], w00, None, MUL
                            )
                        else:
                            wv = w_scalars
                            for j in range(2):
                                vj = wk.tile([128, NPATCH], F32, tag=f"vj{j}")
                                nc.vector.tensor_scalar(
                                    vj[:], sigb[:, :, j, 1],
                                    float(wv[2 * j + 1]),
                                    float(wv[2 * j] + wv[2 * j + 1]),
                                    MUL, ADD,
                                )
                                nc.vector.scalar_tensor_tensor(
                                    t01h[:, 0, :, j], sigb[:, :, j, 0],
                                    float(wv[2 * j]), vj[:], MUL, ADD,
                                )
                        prod = wk.tile([128, NCH, NPATCH, 2], F16, tag="prod")
                        nc.vector.tensor_mul(
                            prod[:], at[:],
                            t01h[:].to_broadcast([128, NCH, NPATCH, 2]),
                        )
                        # acm add off DVE (the global bottleneck) except for
                        # the tail batches, where gpsimd's slow op + the
                        # cross-engine hop would sit on the critical path
                        acm_eng = nc.vector if b >= NB - 2 else nc.gpsimd
                        acm_eng.tensor_add(
                            acm[:, :, 1:NOUT].rearrange(
                                "p k (n u) -> p k n u", u=1
                            ),
                            prod[:, :, :, 0:1], prod[:, :, :, 1:2],
                        )
                    else:
                        # fully general per-channel weights: m_j[c, p] =
                        # w[c,2j]*sig_j0 + w[c,2j+1]*sig_j1 + wsum_j[c]
                        for k in range(NCH):
                            mj = []
                            for j in range(2):
                                u = wk.tile([128, NPATCH], F32, tag=f"u{j}")
                                nc.vector.tensor_scalar_mul(
                                    u[:], sigb[:, :, j, 0],
                                    wq_h[:, k, 2 * j : 2 * j + 1],
                                )
                                m = wk.tile([128, NPATCH], F32, tag=f"m{j}")
                                nc.vector.scalar_tensor_tensor(
                                    m[:], sigb[:, :, j, 1],
                                    wq_h[:, k, 2 * j + 1 : 2 * j + 2],
                                    u[:], MUL, ADD,
                                )
                                m2 = wk.tile([128, NPATCH], F32, tag=f"m2{j}")
                                nc.vector.tensor_scalar(
                                    m2[:], m[:], wq_h[:, k, 4 + j : 5 + j],
                                    None, ADD,
                                )
                                mj.append(m2)
                            p0 = wk.tile([128, NPATCH], F32, tag="p0")
                            nc.vector.tensor_mul(
                                p0[:], at[:, k, :, 0], mj[0][:]
                            )
                            p1 = wk.tile([128, NPATCH], F32, tag="p1")
                            nc.vector.tensor_mul(
                                p1[:], at[:, k, :, 1], mj[1][:]
                            )
                            nc.vector.tensor_add(
                                acm[:, k, 1:NOUT], p0[:], p1[:]
                            )

                # -- pass B: finals + stores for this group --
                pass_b(bs, acms)

    nc.compile()
    return nc


def prepare(x, edge, W_lin, W_out_cls, weights):
    """Host-side prep shared by kernel() and the timing harness: returns
    (w_scalars, in_maps)."""
    x = np.ascontiguousarray(x, dtype=np.float32)
    edge = np.ascontiguousarray(edge, dtype=np.float32)

    # x channel-major: [B, 128, 3, 784]; free order = (chunk, patch, quad),
    # patch = (i, j) row-major, quad = 2*r + cc
    nodes = x[:, 1:, :].reshape(B, HP, 2, HP, 2, C)
    xcm = (
        nodes.transpose(0, 5, 1, 3, 2, 4)        # [b, c, i, j, r, cc]
        .reshape(B, NCH, 128, HW)
        .transpose(0, 2, 1, 3)                   # [b, p, chunk, n]
        .astype(np.float16)
    )
    # edge channel-major fp8 in 4 token-quarters (DMA-CCE accumulated on
    # device), pre-scaled by 784/785 so the device can use a single 1/784
    # sigmoid scale for both the edge and node means
    ecm = np.zeros((B, C, NE), dtype=np.float32)
    ecm[:, :, 0:N] = edge.transpose(0, 2, 1) * (float(HW) / N)
    ecm8 = (
        ecm.reshape(B, NCH, 128, NE)
        .transpose(0, 2, 1, 3)                   # [b, p, chunk, t]
        .astype(ml_dtypes.float8_e4m3)
    )

    wlt = np.asarray(W_lin, dtype=np.float32).T          # [c, c']
    wlt_cm = (
        wlt.reshape(NCH, 128, C).transpose(1, 0, 2).astype(np.float16)
    )
    wct = np.asarray(W_out_cls, dtype=np.float32).T      # [c, co]
    wct_cm = (
        wct.reshape(NCH, 128, CO).transpose(1, 0, 2).astype(np.float16)
    )
    w = np.asarray(weights, dtype=np.float32)
    c_uniform = bool(np.all(w == w[0:1]))
    w_scalars = tuple(float(v) for v in w[0].reshape(4)) if c_uniform else None

    in_maps = []
    for core in range(NCORES):
        sl = slice(core * NB, (core + 1) * NB)
        cls_cm = (
            x[sl, 0, :].T.reshape(NCH, 128, NB)
            .transpose(1, 0, 2)
            .astype(np.float16)
        )
        m = {
            "x": xcm[sl], "edge": ecm8[sl], "wlt": wlt_cm, "wct": wct_cm,
            "cls_cm": np.ascontiguousarray(cls_cm),
        }
        if w_scalars is None:
            wq4 = w.reshape(C, 4)
            wq = np.concatenate(
                [wq4, wq4[:, 0:2].sum(1, keepdims=True),
                 wq4[:, 2:4].sum(1, keepdims=True)], axis=1
            )                                            # [C, 6]
            wq = wq.reshape(NCH, 128, 6).transpose(1, 0, 2)
            m["wq_cm"] = np.ascontiguousarray(wq, dtype=np.float32)
        in_maps.append(m)
    return w_scalars, in_maps


def kernel(x, edge, W_lin, W_out_cls, weights):
    w_scalars, in_maps = prepare(x, edge, W_lin, W_out_cls, weights)
    nc = build_program(w_scalars)
    res = run_bass_kernel_spmd(nc, in_maps, list(range(NCORES)))
    out = np.concatenate(
        [r["out"].astype(np.float32) for r in res.results], axis=0
    )
    return out
